# revision 3
# baseline (speedup 1.0000x reference)
"""Bass/Trainium2 kernel for nn_BasicLstm: 2-layer LSTM (H=512) with
autoregressive output feedback, B=64, F=128 frames, out dim 2.

Strategy: data-parallel over batch across 8 NeuronCores (8 rows each).
Per core, per frame (batch-major layout, gates [8, 2048]):
  - gates = x @ W_ih.T + h @ W_hh.T + b computed with weights as the PE
    *moving* operand (lhsT = x^T / h^T stationary, tiny LDWEIGHTS).
  - Gate blocks reordered [g, i, f, o] so tanh(g)/sigmoid(i) finish first
    and the c-update chain starts before o's matmul completes.
  - Bias + x-part fused: lhsT = x^T augmented to [ctx(2), prev_out(2), 1]
    (row 4 constant 1 multiplies the bias row of the weight matrix).
  - h -> h^T via 4 PE transposes into one PSUM tile + one DVE copy.
  - out^T(t) = W_out.T-chunks @ h1^T chunks (tiny matmuls) + b_out, copied
    directly into frame t+1's x^T rows 2:4 (the feedback path).
Output is accumulated as out^T [2, F*8] and untransposed on the host.
"""
import numpy as np

B, F, H, IN, OUT = 64, 128, 512, 4, 2
NCORES = 8
BL = B // NCORES  # local batch rows per core
G = 4 * H  # 2048 gate rows
NK = H // 128  # 4 contraction chunks per 512

_cache = {}


def build_nc(n_frames=F):
    import concourse.bacc as bacc
    import concourse.bass as bass
    import concourse.mybir as mybir
    import concourse.tile as tile

    fp32 = mybir.dt.float32
    bf16 = mybir.dt.bfloat16
    AF = mybir.ActivationFunctionType
    OP = mybir.AluOpType

    nc = bacc.Bacc(
        "TRN2", target_bir_lowering=False, debug=False, num_devices=NCORES
    )
    NF = n_frames

    d_wx0 = nc.dram_tensor("wx0", [5, G], bf16, kind="ExternalInput")
    d_whh0 = nc.dram_tensor("whh0", [NK, 128, G], bf16, kind="ExternalInput")
    d_w1 = nc.dram_tensor("w1", [2 * NK, 128, G], bf16, kind="ExternalInput")
    d_b1 = nc.dram_tensor("b1", [BL, G], fp32, kind="ExternalInput")
    d_wout = nc.dram_tensor("wout", [NK, 128, OUT], bf16, kind="ExternalInput")
    d_bout = nc.dram_tensor("bout", [OUT, 1], fp32, kind="ExternalInput")
    d_xt = nc.dram_tensor("xt", [5, NF * BL], bf16, kind="ExternalInput")
    d_ident = nc.dram_tensor("ident", [BL, BL], fp32, kind="ExternalInput")
    d_y = nc.dram_tensor("y", [OUT, NF * BL], fp32, kind="ExternalOutput")

    # Raw SBUF tensors (persistent, Tile still tracks accesses).
    wx0_sb = nc.alloc_sbuf_tensor("wx0_sb", [5, G], bf16)
    whh0_sb = nc.alloc_sbuf_tensor("whh0_sb", [128, NK * G], bf16)
    w1_sb = nc.alloc_sbuf_tensor("w1_sb", [128, 2 * NK * G], bf16)
    b1_sb = nc.alloc_sbuf_tensor("b1_sb", [BL, G], fp32)
    wout_sb = nc.alloc_sbuf_tensor("wout_sb", [128, NK * OUT], bf16)
    bout_sb = nc.alloc_sbuf_tensor("bout_sb", [OUT, 1], fp32)
    xt_sb = nc.alloc_sbuf_tensor("xt_sb", [5, NF * BL], bf16)
    ident_sb = nc.alloc_sbuf_tensor("ident_sb", [BL, BL], fp32)
    ones_sb = nc.alloc_sbuf_tensor("ones_sb", [1, BL], bf16)
    outT_sb = nc.alloc_sbuf_tensor("outT_sb", [OUT, NF * BL], fp32)

    # gate chunk order [g, i, f, o] -> activation per N-chunk
    CHUNK_FUNC = [AF.Tanh, AF.Sigmoid, AF.Sigmoid, AF.Sigmoid]

    with tile.TileContext(nc) as tc:
        with tc.tile_pool(name="psum_g", bufs=3, space="PSUM") as pg, \
             tc.tile_pool(name="psum_tp", bufs=1, space="PSUM") as ptp, \
             tc.tile_pool(name="psum_o", bufs=1, space="PSUM") as po, \
             tc.tile_pool(name="sb", bufs=3) as sb:

            # ---- one-time loads ----
            nc.sync.dma_start(wx0_sb[:], d_wx0[:])
            for k in range(NK):
                nc.sync.dma_start(whh0_sb[:, k * G:(k + 1) * G], d_whh0[k])
            for k in range(2 * NK):
                nc.sync.dma_start(w1_sb[:, k * G:(k + 1) * G], d_w1[k])
            nc.sync.dma_start(b1_sb[:], d_b1[:])
            for k in range(NK):
                nc.sync.dma_start(wout_sb[:, k * OUT:(k + 1) * OUT], d_wout[k])
            nc.sync.dma_start(bout_sb[:], d_bout[:])
            nc.sync.dma_start(xt_sb[:], d_xt[:])
            nc.sync.dma_start(ident_sb[:], d_ident[:])
            nc.vector.memset(ones_sb[:], 1.0)

            hT = {0: None, 1: None}   # [128, NK*BL] transposed hidden state
            c = {0: None, 1: None}    # [BL, H] cell state

            for t in range(NF):
                xcol = slice(t * BL, (t + 1) * BL)

                for layer in (0, 1):
                    # ---- gates matmuls, 4 N-chunks of 512 ----
                    acts = []
                    for n in range(4):
                        gp = pg.tile([BL, 512], fp32, name=f"gp{layer}", tag=f"gp{layer}")
                        ncol = slice(n * 512, (n + 1) * 512)
                        if layer == 0:
                            # hh0 first (independent of the feedback chain),
                            # x-part + bias (K=5) joins last.
                            if t > 0:
                                for k in range(NK):
                                    nc.tensor.matmul(
                                        gp[:],
                                        hT[0][:, k * BL:(k + 1) * BL],
                                        whh0_sb[:, k * G + n * 512: k * G + (n + 1) * 512],
                                        start=(k == 0), stop=False,
                                    )
                            nc.tensor.matmul(
                                gp[:], xt_sb[0:5, xcol], wx0_sb[0:5, ncol],
                                start=(t == 0), stop=True,
                            )
                        else:
                            if t > 0:
                                # hh1-part (h1(t-1)): available early
                                for k in range(NK):
                                    nc.tensor.matmul(
                                        gp[:],
                                        hT[1][:, k * BL:(k + 1) * BL],
                                        w1_sb[:, (NK + k) * G + n * 512: (NK + k) * G + (n + 1) * 512],
                                        start=(k == 0), stop=False,
                                    )
                            # ih1-part (h0(t))
                            for k in range(NK):
                                nc.tensor.matmul(
                                    gp[:],
                                    hT[0][:, k * BL:(k + 1) * BL],
                                    w1_sb[:, k * G + n * 512: k * G + (n + 1) * 512],
                                    start=(t == 0 and k == 0), stop=(k == NK - 1),
                                )
                        # activation for this chunk (PSUM -> SBUF);
                        # layer1 bias joins via DVE add (keeps PE free)
                        av = sb.tile([BL, 512], fp32, name=f"act{layer}_{n}",
                                     tag=f"act{layer}_{n}")
                        if layer == 1:
                            avp = sb.tile([BL, 512], fp32, name=f"ab{n}",
                                          tag=f"ab{n}")
                            nc.vector.tensor_tensor(
                                avp[:], gp[:], b1_sb[:, ncol], OP.add)
                            asrc = avp
                        else:
                            asrc = gp
                        if n == 3:
                            nc.scalar.activation(
                                av[:, 0:256], asrc[:, 0:256], CHUNK_FUNC[n])
                            nc.scalar.activation(
                                av[:, 256:512], asrc[:, 256:512], CHUNK_FUNC[n])
                        else:
                            nc.scalar.activation(av[:], asrc[:], CHUNK_FUNC[n])
                        acts.append(av)

                    tg, si, sf, so = acts
                    # ---- c update (half-width pipelined stages) ----
                    c_new = sb.tile([BL, H], fp32, name=f"c{layer}", tag=f"c{layer}")
                    tc_t = sb.tile([BL, H], fp32, name=f"tc{layer}", tag=f"tc{layer}")
                    h_bm = sb.tile([BL, H], fp32, name=f"h{layer}", tag=f"h{layer}")
                    if t == 0:
                        for hs in (slice(0, 256), slice(256, 512)):
                            nc.vector.tensor_tensor(
                                c_new[:, hs], si[:, hs], tg[:, hs], OP.mult)
                            nc.scalar.activation(
                                tc_t[:, hs], c_new[:, hs], AF.Tanh)
                            nc.vector.tensor_tensor(
                                h_bm[:, hs], so[:, hs], tc_t[:, hs], OP.mult)
                    else:
                        m1 = sb.tile([BL, H], fp32, name=f"m1_{layer}", tag=f"m1_{layer}")
                        m2 = sb.tile([BL, H], fp32, name=f"m2_{layer}", tag=f"m2_{layer}")
                        for hs in (slice(0, 256), slice(256, 512)):
                            nc.vector.tensor_tensor(
                                m1[:, hs], si[:, hs], tg[:, hs], OP.mult)
                            nc.vector.tensor_tensor(
                                m2[:, hs], sf[:, hs], c[layer][:, hs], OP.mult)
                            nc.vector.tensor_tensor(
                                c_new[:, hs], m1[:, hs], m2[:, hs], OP.add)
                            nc.scalar.activation(
                                tc_t[:, hs], c_new[:, hs], AF.Tanh)
                            nc.vector.tensor_tensor(
                                h_bm[:, hs], so[:, hs], tc_t[:, hs], OP.mult)
                    c[layer] = c_new

                    # ---- transpose h [8,512] -> h^T [128, 4*8] ----
                    tp = ptp.tile([128, NK * BL], fp32, name="tp", tag="tp")
                    for k in range(NK):
                        nc.tensor.transpose(
                            tp[:, k * BL:(k + 1) * BL],
                            h_bm[:, k * 128:(k + 1) * 128],
                            ident_sb[:],
                        )
                    hT_new = sb.tile([128, NK * BL], bf16, name=f"hT{layer}",
                                     tag=f"hT{layer}")
                    nc.vector.tensor_copy(hT_new[:], tp[:])
                    hT[layer] = hT_new

                # ---- out^T(t) = W_out^T-chunks @ h1^T + b_out ----
                ot = po.tile([OUT, BL], fp32, name="ot", tag="ot")
                for k in range(NK):
                    nc.tensor.matmul(
                        ot[:],
                        wout_sb[:, k * OUT:(k + 1) * OUT],
                        hT[1][:, k * BL:(k + 1) * BL],
                        start=(k == 0), stop=(k == NK - 1),
                    )
                if t + 1 < NF:
                    nc.vector.tensor_scalar_add(
                        xt_sb[0:2, (t + 1) * BL:(t + 2) * BL], ot[:], bout_sb[:])
                nc.vector.tensor_scalar_add(outT_sb[:, xcol], ot[:], bout_sb[:])

            nc.sync.dma_start(d_y[:], outT_sb[:])

    nc.compile()
    return nc


def _prep_inputs(inputs, W_ih0, W_hh0, b_ih0, b_hh0, W_ih1, W_hh1, b_ih1,
                 b_hh1, W_out, b_out, n_frames=F):
    """Build per-core input maps (numpy only)."""
    NF = n_frames
    import ml_dtypes
    f32 = np.float32
    bf = ml_dtypes.bfloat16
    perm = np.r_[2 * H:3 * H, 0:H, H:2 * H, 3 * H:4 * H]  # [g,i,f,o]

    # x^T row order: [prev_out(2), ctx(2), ones(1)] — prev_out first so the
    # per-frame feedback copy (out^T partitions 0:2) is lane-aligned.
    wx0 = np.concatenate(
        [W_ih0[perm, 2:4].T, W_ih0[perm, 0:2].T,
         (b_ih0 + b_hh0)[perm][None, :]], axis=0).astype(bf)
    whh0 = np.ascontiguousarray(W_hh0[perm].T.reshape(NK, 128, G)).astype(bf)
    w1 = np.concatenate([W_ih1[perm].T, W_hh1[perm].T], axis=0)
    w1 = np.ascontiguousarray(w1.reshape(2 * NK, 128, G)).astype(bf)
    b1 = np.broadcast_to((b_ih1 + b_hh1)[perm][None, :], (BL, 4 * H)).astype(f32)
    b1 = np.ascontiguousarray(b1)
    wout = np.ascontiguousarray(W_out.T.reshape(NK, 128, OUT)).astype(bf)
    bout = b_out.reshape(OUT, 1).astype(f32)
    ident = np.eye(BL, dtype=f32)

    in_maps = []
    for cid in range(NCORES):
        bs = slice(cid * BL, (cid + 1) * BL)
        xt = np.zeros((5, NF * BL), bf)
        xt[4] = 1.0
        for t in range(NF):
            xt[2:4, t * BL:(t + 1) * BL] = inputs[bs, t, 0:2].T
        xt[0:2, 0:BL] = inputs[bs, 0, 2:4].T
        in_maps.append({
            "wx0": wx0, "whh0": whh0, "w1": w1, "b1": b1, "wout": wout,
            "bout": bout, "xt": xt, "ident": ident,
        })
    return in_maps


def run(inputs, n_frames=F, trace=False, **params):
    from concourse import bass_utils

    key = n_frames
    if key not in _cache:
        _cache[key] = build_nc(n_frames)
    nc = _cache[key]
    in_maps = _prep_inputs(inputs, n_frames=n_frames, **params)
    res = bass_utils.run_bass_kernel_spmd(
        nc, in_maps, core_ids=list(range(NCORES)), trace=trace
    )
    NF = n_frames
    out = np.zeros((B, NF, OUT), np.float32)
    for cid in range(NCORES):
        y = res.results[cid]["y"]  # [2, NF*BL]
        out[cid * BL:(cid + 1) * BL] = y.reshape(OUT, NF, BL).transpose(2, 1, 0)
    return out, res


def kernel(**inputs):
    inputs = {k: np.asarray(v) for k, v in inputs.items()}
    out, _ = run(**inputs)
    return out

